# revision 6
# baseline (speedup 1.0000x reference)
"""Masked-softmax complementary-label loss on 8 Trainium2 NeuronCores.

Strategy (pure data parallel, hardcoded for B=32768, C=1000, K=10):
  - Shard batch across 8 cores (4096 rows each).
  - Each core streams its [4096, 1000] f32 logit shard through SBUF and
    computes per-row Z = sum_c exp(x[r, c]) using the scalar engine's
    exp activation with free-dim accumulation (memory-bound part).
  - Host gathers the 10 complementary-label logits per row (tiny),
    dedups duplicate labels, and finishes the per-row loss:
        S   = sum_k w_k * exp(g_k)          (w: first-occurrence weights)
        p_nc = (Z - S) / Z                  (probability mass not in set)
        loss = -log(p_nc + 1e-7)
        out  = mean(scale * loss),  scale = (C-1)/(C - num_comp)
"""

import numpy as np

B = 32768
C = 1000
K = 10
NCORES = 8
B_LOCAL = B // NCORES  # 4096
P = 128
NT = B_LOCAL // P  # 32 row-tiles of 128 rows per core
TPB = 2  # row-tiles per DMA (1 MB per transfer)
EPS = 1e-7

_PROG_CACHE = {}


def _build_program():
    """Build the single-core Bass program (SPMD across 8 cores).

    Raw Bass (no TileContext): this toolchain's walrus rejects instructions
    with more than a couple of embedded sync-wait commands, which Tile's
    scheduler and tail drain freely emit. With manual semaphores every wait
    is its own sequencer instruction, so there is no such limit.

    Layout: the whole 16 MB shard stays resident in SBUF (125 KB of the
    192 KB partition budget), so load DMAs have no WAR hazards at all.
    """
    import concourse.bass as bass
    from concourse import mybir

    nc = bass.Bass(
        "TRN2", target_bir_lowering=False, debug=False, num_devices=NCORES
    )
    x = nc.dram_tensor(
        "x", [B_LOCAL, C], mybir.dt.float32, kind="ExternalInput"
    ).ap()
    z = nc.dram_tensor(
        "z", [P, NT], mybir.dt.float32, kind="ExternalOutput"
    ).ap()
    x3 = x.rearrange("(n p) c -> n p c", p=P)  # [NT, P, C]
    nblk = NT // TPB

    with (
        nc.sbuf_tensor([P, NT * C], mybir.dt.float32) as xbuf,
        nc.sbuf_tensor([P, NT], mybir.dt.float32) as ztile,
        nc.semaphore() as in_sem,
        nc.semaphore() as act_sem,
        nc.semaphore() as out_sem,
        nc.Block() as block,
    ):

        @block.sync
        def _(sp):
            for blk in range(nblk):
                src = x3[blk * TPB : (blk + 1) * TPB].rearrange("n p c -> p n c")
                dst = xbuf[:, blk * TPB * C : (blk + 1) * TPB * C].rearrange(
                    "p (n c) -> p n c", c=C
                )
                sp.dma_start(dst, src).then_inc(in_sem, 16)
            sp.wait_ge(act_sem, NT)
            sp.dma_start(z, ztile[:]).then_inc(out_sem, 16)
            sp.wait_ge(out_sem, 16)

        @block.scalar
        def _(act):
            for blk in range(nblk):
                act.wait_ge(in_sem, (blk + 1) * 16)
                for j in range(TPB):
                    i = blk * TPB + j
                    sub = xbuf[:, i * C : (i + 1) * C]
                    act.activation(
                        sub,
                        sub,
                        mybir.ActivationFunctionType.Exp,
                        accum_out=ztile[:, i : i + 1],
                    ).then_inc(act_sem, 1)

    return nc


def _get_program():
    if "nc" not in _PROG_CACHE:
        _PROG_CACHE["nc"] = _build_program()
    return _PROG_CACHE["nc"]


def run_device(outputs_np, trace=False, trace_kwargs=None):
    """Run the Bass kernel on 8 cores; returns (Z[B] float32, BassKernelResults)."""
    from concourse.bass_utils import run_bass_kernel_spmd

    nc = _get_program()
    in_maps = [
        {"x": np.ascontiguousarray(outputs_np[r * B_LOCAL : (r + 1) * B_LOCAL])}
        for r in range(NCORES)
    ]
    kw = {}
    if trace:
        kw["trace"] = True
        if trace_kwargs:
            kw["trace_kwargs"] = trace_kwargs
    res = run_bass_kernel_spmd(nc, in_maps, list(range(NCORES)), **kw)
    zs = [np.asarray(res.results[r]["z"]) for r in range(NCORES)]
    # z[p, i] corresponds to shard row i*P + p
    Z = np.concatenate([z.T.reshape(-1) for z in zs])  # [B]
    return Z, res


def _host_label_prep(outputs_np, labels_np):
    """Dedup weights, gathered logits, and per-row scale from the labels."""
    labels = labels_np.astype(np.int64)
    valid = labels != -1  # [B, K]
    num_comp = valid.sum(axis=1)  # [B]
    # first-occurrence mask: entry k is a dup if some j < k holds same value
    eq = labels[:, :, None] == labels[:, None, :]  # [B, K, K]
    earlier = np.arange(K)[None, :] < np.arange(K)[:, None]  # [K, K], (k, j): j<k
    is_dup = (eq & earlier[None, :, :]).any(axis=2)  # [B, K]
    w = valid & ~is_dup  # [B, K] bool
    safe = np.where(valid, labels, 0)
    g = outputs_np[np.arange(B)[:, None], safe]  # [B, K] f32 gathered logits
    return w, g, num_comp


def finish_loss(Z, w, g, num_comp):
    S = np.where(w, np.exp(g.astype(np.float64)), 0.0).sum(axis=1)  # [B]
    Z64 = Z.astype(np.float64)
    p_nc = (Z64 - S) / Z64
    loss = -np.log(p_nc + EPS)
    scale = (C - 1) / (C - num_comp.astype(np.float64))
    return np.asarray((scale * loss).mean(), dtype=np.float32)


def kernel(**inputs):
    outputs_np = np.ascontiguousarray(
        np.asarray(inputs["outputs"], dtype=np.float32)
    )
    labels_np = np.asarray(inputs["complementary_labels"])
    assert outputs_np.shape == (B, C)
    assert labels_np.shape == (B, K)

    w, g, num_comp = _host_label_prep(outputs_np, labels_np)
    Z, _ = run_device(outputs_np)
    return finish_loss(Z, w, g, num_comp)
